# revision 4
# baseline (speedup 1.0000x reference)
"""BigBird sparse attention on 8 Trainium2 NeuronCores.

Sharding: batch*heads = 64 (b,h) pairs, 8 pairs per core (pure data
parallel, no collectives). Host-side prep transposes Q/K to [D, T],
gathers the random keys/values, and augments V with a ones column so the
softmax denominator falls out of the attention*V matmul as row D.

Per (b,h) pair on-core:
  Stage A (global queries, tokens 0..63): S_g^T chunks [128k, 64q] via
    matmul(lhsT=K^T chunk, rhs=Qg^T), exp on ScalarE, then accumulated
    AV matmuls with V_aug chunks as stationary producing out_g^T [65, 64]
    (row 64 = denominator). PE-transpose back, normalize on VectorE.
  Stage B (block queries): chunks of up to 8 blocks (512 queries).
    Scores are computed keys-on-partitions: local per block [64, 64],
    global [64, CQ], random [128+64, CQ]. exp on ScalarE -> E tiles in
    SBUF. AV: out^T [65, CQ] accumulating matmuls with V_aug stationary
    (global/random batched over the whole chunk, local per block).
    PE-transpose to [q, 65], normalize by reciprocal of col 64, DMA out.

Softmax skips max-subtraction: scores/sqrt(D) are ~N(0,1) for randn
inputs, so exp stays comfortably inside fp32 range and normalization
cancels the shift exactly in exact math.

Precision knobs: the large-N matmuls (N>=256 moving) can run as
hardware float32r (1 cycle/row vs 4 for fp32). float32r data must be
produced as float32r end-to-end (DRAM declaration + ACT output dtype),
with fp32 bitcast views at fp32-consuming sites.
"""

import numpy as np

B, T, H, D = 4, 4096, 16, 64
BS, G, R = 64, 64, 192
NCORE = 8
BH = B * H
NPAIR = BH // NCORE          # 8 pairs per core
NB = (T - G) // BS           # 63 local blocks
INV_SCALE = float(D) ** -0.5
DA = D + 1                   # V augmented with ones column
NKC = T // 128               # 32 key chunks of 128

# matmul input precision for the large-N (>=256 moving) matmuls:
#   "f32"  - exact fp32 (4 cycles/row)
#   "f32r" - hardware fp32r (1 cycle/row at N>=256, reduced precision)
BIG_MM = "f32r"
# precision for the small-N (64/65 moving) matmuls: "f32" or "f32r"
SMALL_MM = "f32"

_PROGRAM_CACHE = {}


def _body(ctx, tc, qT, kT, krT, vch, vr, vb, out):
    import concourse.mybir as mybir
    from concourse.masks import make_identity

    nc = tc.nc
    f32 = mybir.dt.float32
    f32r = mybir.dt.float32r
    DT_BIG = f32r if BIG_MM == "f32r" else f32
    DT_SMALL = f32r if SMALL_MM == "f32r" else f32
    # tiles consumed by both big- and small-N matmuls
    DT_MIX = f32r if (BIG_MM == "f32r" or SMALL_MM == "f32r") else f32
    EXP = mybir.ActivationFunctionType.Exp

    def as_dt(ap, dt):
        return ap if ap.dtype == dt else ap.bitcast(dt)

    consts = ctx.enter_context(tc.tile_pool(name="consts", bufs=1))
    pin = ctx.enter_context(tc.tile_pool(name="pin", bufs=2))
    pe = ctx.enter_context(tc.tile_pool(name="pe", bufs=3))
    pegt = ctx.enter_context(tc.tile_pool(name="pegt", bufs=2))
    ps1 = ctx.enter_context(tc.tile_pool(name="ps1", bufs=1, space="PSUM"))
    ps2 = ctx.enter_context(tc.tile_pool(name="ps2", bufs=2, space="PSUM"))

    ident = consts.tile([128, 128], f32)
    make_identity(nc, ident)

    # block chunking: 7 chunks of 8 blocks + 1 chunk of 7 blocks
    chunks = []
    n0 = 0
    while n0 < NB:
        nb = min(8, NB - n0)
        chunks.append((n0, nb))
        n0 += nb

    for p in range(NPAIR):
        # ---- load pair inputs ----
        qt = pin.tile([D, T], DT_MIX, tag="qT")
        kt = pin.tile([D, T], DT_MIX, tag="kT")
        krt = pin.tile([D, R], DT_BIG, tag="krT")
        vc = pin.tile([128, NKC, DA], DT_MIX, tag="vch")
        vr1 = pin.tile([128, DA], DT_BIG, tag="vr1")
        vr2 = pin.tile([R - 128, DA], DT_BIG, tag="vr2")
        vbt = pin.tile([BS, NB, DA], DT_SMALL, tag="vb")
        nc.sync.dma_start(out=qt, in_=as_dt(qT[p], DT_MIX))
        nc.sync.dma_start(out=kt, in_=as_dt(kT[p], DT_MIX))
        nc.sync.dma_start(out=krt, in_=as_dt(krT[p], DT_BIG))
        nc.sync.dma_start(out=vc, in_=as_dt(vch[p], DT_MIX))
        nc.sync.dma_start(out=vr1, in_=as_dt(vr[p, 0:128, :], DT_BIG))
        nc.sync.dma_start(out=vr2, in_=as_dt(vr[p, 128:R, :], DT_BIG))
        nc.sync.dma_start(out=vbt, in_=as_dt(vb[p], DT_SMALL))

        # ---- Stage A: global queries (tokens 0..G) ----
        egt = pegt.tile([128, NKC, G], DT_SMALL, tag="egt")
        for kc in range(NKC):
            sgt = ps2.tile([128, G], f32, tag="big")
            nc.tensor.matmul(
                sgt,
                as_dt(kt[:, 128 * kc : 128 * (kc + 1)], DT_SMALL),
                as_dt(qt[:, 0:G], DT_SMALL),
                start=True,
                stop=True,
            )
            nc.scalar.activation(egt[:, kc, :], sgt, EXP, scale=INV_SCALE)
        outg = ps2.tile([DA, G], f32, tag="pt")
        for kc in range(NKC):
            nc.tensor.matmul(
                outg,
                as_dt(vc[:, kc, :], DT_SMALL),
                egt[:, kc, :],
                start=(kc == 0),
                stop=(kc == NKC - 1),
            )
        outg_sb = pe.tile([DA, G], f32, tag="poutsA")
        nc.vector.tensor_copy(outg_sb, outg)
        outg2 = ps2.tile([G, DA], f32, tag="pt")
        nc.tensor.transpose(outg2, outg_sb, ident[:DA, :DA])
        recg = pe.tile([G, 1], f32, tag="recipA")
        nc.vector.reciprocal(recg, outg2[:, D : D + 1])
        outg_n = pe.tile([G, D], f32, tag="outnA")
        nc.vector.tensor_scalar_mul(outg_n, outg2[:, 0:D], recg)
        nc.sync.dma_start(out=out[p, 0:G, :], in_=outg_n)

        # ---- Stage B: block queries ----
        for n0, nb in chunks:
            cq = BS * nb
            qoff = G + BS * n0  # first query token of this chunk

            ploc = ps1.tile([BS, 512], f32, tag="s_loc")
            pglo = ps1.tile([G, 512], f32, tag="s_glo")
            prnd = ps1.tile([128, 512], f32, tag="s_rnd")
            prnd2 = ps1.tile([R - 128, 512], f32, tag="s_rnd2")

            for j in range(nb):
                n = n0 + j
                koff = G + BS * n
                nc.tensor.matmul(
                    ploc[:, BS * j : BS * (j + 1)],
                    as_dt(kt[:, koff : koff + BS], DT_SMALL),
                    as_dt(qt[:, koff : koff + BS], DT_SMALL),
                    start=(j == 0),
                    stop=(j == nb - 1),
                )
            nc.tensor.matmul(
                pglo[:, 0:cq],
                as_dt(kt[:, 0:G], DT_BIG),
                as_dt(qt[:, qoff : qoff + cq], DT_BIG),
                start=True,
                stop=True,
            )
            nc.tensor.matmul(
                prnd[:, 0:cq],
                as_dt(krt[:, 0:128], DT_BIG),
                as_dt(qt[:, qoff : qoff + cq], DT_BIG),
                start=True,
                stop=True,
            )
            nc.tensor.matmul(
                prnd2[:, 0:cq],
                as_dt(krt[:, 128:R], DT_BIG),
                as_dt(qt[:, qoff : qoff + cq], DT_BIG),
                start=True,
                stop=True,
            )

            eloc = pe.tile([BS, 512], DT_SMALL, tag="eloc")
            eglo = pe.tile([G, 512], DT_BIG, tag="eglo")
            ernd = pe.tile([128, 512], DT_BIG, tag="ernd")
            ernd2 = pe.tile([R - 128, 512], DT_BIG, tag="ernd2")
            nc.scalar.activation(eloc[:, 0:cq], ploc[:, 0:cq], EXP, scale=INV_SCALE)
            nc.scalar.activation(eglo[:, 0:cq], pglo[:, 0:cq], EXP, scale=INV_SCALE)
            nc.scalar.activation(ernd[:, 0:cq], prnd[:, 0:cq], EXP, scale=INV_SCALE)
            nc.scalar.activation(ernd2[:, 0:cq], prnd2[:, 0:cq], EXP, scale=INV_SCALE)

            pout = ps2.tile([DA, 512], f32, tag="big")
            nc.tensor.matmul(
                pout[:, 0:cq],
                as_dt(vc[0:G, 0, :], DT_BIG),
                eglo[:, 0:cq],
                start=True,
                stop=False,
            )
            nc.tensor.matmul(
                pout[:, 0:cq],
                as_dt(vr1, DT_BIG),
                ernd[:, 0:cq],
                start=False,
                stop=False,
            )
            nc.tensor.matmul(
                pout[:, 0:cq],
                as_dt(vr2, DT_BIG),
                ernd2[:, 0:cq],
                start=False,
                stop=False,
            )
            for j in range(nb):
                n = n0 + j
                nc.tensor.matmul(
                    pout[:, BS * j : BS * (j + 1)],
                    as_dt(vbt[:, n, :], DT_SMALL),
                    eloc[:, BS * j : BS * (j + 1)],
                    start=False,
                    stop=(j == nb - 1),
                )

            pout_sb = pe.tile([DA, 512], f32, tag="pouts")
            nc.vector.tensor_copy(pout_sb[:, 0:cq], pout[:, 0:cq])

            nt = (cq + 127) // 128
            pt = ps2.tile([128, nt * DA], f32, tag="pt")
            for t in range(nt):
                w = min(128, cq - 128 * t)
                nc.tensor.matmul(
                    pt[0:w, DA * t : DA * (t + 1)],
                    pout_sb[:, 128 * t : 128 * t + w],
                    ident[:DA, :DA],
                    is_transpose=True,
                    start=True,
                    stop=True,
                )
            for t in range(nt):
                w = min(128, cq - 128 * t)
                rec = pe.tile([128, 1], f32, tag="recip")
                nc.vector.reciprocal(rec[0:w], pt[0:w, DA * t + D : DA * (t + 1)])
                outn = pe.tile([128, D], f32, tag="outn")
                nc.vector.tensor_scalar_mul(
                    outn[0:w], pt[0:w, DA * t : DA * t + D], rec[0:w]
                )
                r0 = qoff + 128 * t
                nc.sync.dma_start(out=out[p, r0 : r0 + w, :], in_=outn[0:w])


def _build_program():
    from contextlib import ExitStack

    import concourse.bacc as bacc
    import concourse.mybir as mybir
    import concourse.tile as tile

    f32 = mybir.dt.float32
    f32r = mybir.dt.float32r
    DT_BIG = f32r if BIG_MM == "f32r" else f32
    DT_SMALL = f32r if SMALL_MM == "f32r" else f32
    DT_MIX = f32r if (BIG_MM == "f32r" or SMALL_MM == "f32r") else f32

    nc = bacc.Bacc(
        "TRN2", target_bir_lowering=False, debug=False, num_devices=NCORE
    )
    qT = nc.dram_tensor("qT", [NPAIR, D, T], DT_MIX, kind="ExternalInput").ap()
    kT = nc.dram_tensor("kT", [NPAIR, D, T], DT_MIX, kind="ExternalInput").ap()
    krT = nc.dram_tensor("krT", [NPAIR, D, R], DT_BIG, kind="ExternalInput").ap()
    vch = nc.dram_tensor(
        "vch", [NPAIR, 128, NKC, DA], DT_MIX, kind="ExternalInput"
    ).ap()
    vr = nc.dram_tensor("vr", [NPAIR, R, DA], DT_BIG, kind="ExternalInput").ap()
    vb = nc.dram_tensor("vb", [NPAIR, BS, NB, DA], DT_SMALL, kind="ExternalInput").ap()
    out = nc.dram_tensor("out", [NPAIR, T, D], mybir.dt.float32, kind="ExternalOutput").ap()

    with tile.TileContext(nc) as tc:
        with ExitStack() as ctx:
            _body(ctx, tc, qT, kT, krT, vch, vr, vb, out)
    nc.compile()
    return nc


def get_program():
    key = (BIG_MM, SMALL_MM)
    if key not in _PROGRAM_CACHE:
        _PROGRAM_CACHE[key] = _build_program()
    return _PROGRAM_CACHE[key]


def prep_inputs(q, k, v, rand_idx):
    """Host-side shard + layout prep. Returns list of per-core input dicts."""
    idx = np.asarray(rand_idx).astype(np.int64)
    qp = np.ascontiguousarray(q.transpose(0, 2, 3, 1)).reshape(BH, D, T)
    kp = np.ascontiguousarray(k.transpose(0, 2, 3, 1)).reshape(BH, D, T)
    krTp = np.ascontiguousarray(kp[:, :, idx])  # [BH, D, R]
    vp = np.ascontiguousarray(v.transpose(0, 2, 1, 3)).reshape(BH, T, D)
    v_aug = np.concatenate([vp, np.ones((BH, T, 1), np.float32)], axis=2)
    vchp = np.ascontiguousarray(
        v_aug.reshape(BH, NKC, 128, DA).transpose(0, 2, 1, 3)
    )  # [BH, 128, NKC, DA]
    vrp = np.ascontiguousarray(v_aug[:, idx, :])  # [BH, R, DA]
    vbp = np.ascontiguousarray(
        v_aug[:, G:, :].reshape(BH, NB, BS, DA).transpose(0, 2, 1, 3)
    )  # [BH, BS, NB, DA]

    in_maps = []
    for c in range(NCORE):
        s = slice(c * NPAIR, (c + 1) * NPAIR)
        in_maps.append(
            {
                "qT": np.ascontiguousarray(qp[s]),
                "kT": np.ascontiguousarray(kp[s]),
                "krT": np.ascontiguousarray(krTp[s]),
                "vch": np.ascontiguousarray(vchp[s]),
                "vr": np.ascontiguousarray(vrp[s]),
                "vb": np.ascontiguousarray(vbp[s]),
            }
        )
    return in_maps


def assemble_output(results):
    """[8 cores] x {"out": [NPAIR, T, D]} -> [B, T, H, D]"""
    full = np.concatenate([r["out"] for r in results], axis=0)  # [BH, T, D]
    return np.ascontiguousarray(
        full.reshape(B, H, T, D).transpose(0, 2, 1, 3)
    )


def kernel(q, k, v, rand_idx, _trace=False):
    from concourse.bass_utils import run_bass_kernel_spmd

    nc = get_program()
    in_maps = prep_inputs(
        np.asarray(q, dtype=np.float32),
        np.asarray(k, dtype=np.float32),
        np.asarray(v, dtype=np.float32),
        rand_idx,
    )
    res = run_bass_kernel_spmd(nc, in_maps, list(range(NCORE)), trace=_trace)
    out = assemble_output(res.results)
    if _trace:
        return out, res
    return out


# revision 8
# speedup vs baseline: 1.4908x; 1.4908x over previous
"""BigBird sparse attention on 8 Trainium2 NeuronCores.

Sharding: batch*heads = 64 (b,h) pairs, 8 pairs per core (pure data
parallel, no collectives). Host-side prep transposes Q/K to [D, T],
gathers the random keys/values, and augments V with a ones column so the
softmax denominator falls out of the attention*V matmul as row D.

Per (b,h) pair on-core:
  Stage A (global queries, tokens 0..63): S_g^T chunks [128k, 64q] via
    matmul(lhsT=K^T chunk, rhs=Qg^T), exp on ScalarE, then accumulated
    AV matmuls with V_aug chunks as stationary producing out_g^T [65, 64]
    (row 64 = denominator). PE-transpose back, normalize on VectorE.
  Stage B (block queries): chunks of up to 8 blocks (512 queries).
    Scores are computed keys-on-partitions: local per block [64, 64],
    global [64, CQ], random [128+64, CQ]. exp on ScalarE -> E tiles in
    SBUF. AV: out^T [65, CQ] accumulating matmuls with V_aug stationary
    (global/random batched over the whole chunk, local per block).
    PE-transpose to [q, 65], normalize by reciprocal of col 64, DMA out.

Softmax skips max-subtraction: scores/sqrt(D) are ~N(0,1) for randn
inputs, so exp stays comfortably inside fp32 range and normalization
cancels the shift exactly in exact math.

Precision knobs BIG_MM / SMALL_MM select matmul input dtype per class.
float32r data must be produced as float32r end-to-end; bf16 operands are
prepared host-side and exp() writes bf16 E tiles directly.
"""

import numpy as np

B, T, H, D = 4, 4096, 16, 64
BS, G, R = 64, 64, 192
NCORE = 8
BH = B * H
NPAIR = BH // NCORE          # 8 pairs per core
NB = (T - G) // BS           # 63 local blocks
INV_SCALE = float(D) ** -0.5
DA = D + 1                   # V augmented with ones column
NKC = T // 128               # 32 key chunks of 128

# matmul input precision for the large-N (>=256 moving) matmuls:
#   "f32"  - exact fp32 (4 cycles/row)
#   "f32r" - hardware fp32r (reduced precision, ~2 passes on HW)
#   "bf16" - bfloat16 inputs, fp32 accumulate (1 cycle/row, single pass)
BIG_MM = "bf16"
# precision for the small-N (64/65 moving) matmuls: "f32", "f32r" or "bf16"
SMALL_MM = "bf16"

_PROGRAM_CACHE = {}


def _dt32_main(mybir):
    """dtype used for the 32-bit DRAM/SBUF copies."""
    if "f32r" in (BIG_MM, SMALL_MM):
        return mybir.dt.float32r
    return mybir.dt.float32


def _body(ctx, tc, d32, d16, out):
    import concourse.mybir as mybir
    from concourse.masks import make_identity

    nc = tc.nc
    f32 = mybir.dt.float32
    f32r = mybir.dt.float32r
    bf16 = mybir.dt.bfloat16
    DT32 = _dt32_main(mybir)
    DT = {"f32": f32, "f32r": f32r, "bf16": bf16}
    DT_BIG = DT[BIG_MM]
    DT_SMALL = DT[SMALL_MM]
    EXP = mybir.ActivationFunctionType.Exp

    def as_dt(ap, dt):
        return ap if ap.dtype == dt else ap.bitcast(dt)

    consts = ctx.enter_context(tc.tile_pool(name="consts", bufs=1))
    pin = ctx.enter_context(tc.tile_pool(name="pin", bufs=2))
    pe = ctx.enter_context(tc.tile_pool(name="pe", bufs=3))
    pegt = ctx.enter_context(tc.tile_pool(name="pegt", bufs=2))
    ps1 = ctx.enter_context(tc.tile_pool(name="ps1", bufs=1, space="PSUM"))
    ps2 = ctx.enter_context(tc.tile_pool(name="ps2", bufs=2, space="PSUM"))

    ident = consts.tile([128, 128], f32)
    make_identity(nc, ident)

    needs = {
        "qT": {DT_BIG, DT_SMALL},
        "kT": {DT_BIG, DT_SMALL},
        "krT": {DT_BIG},
        "vr1": {DT_BIG},
        "vr2": {DT_BIG},
        "vch": {DT_BIG, DT_SMALL},
        "vb": {DT_SMALL},
    }
    shapes = {
        "qT": [D, T],
        "kT": [D, T],
        "krT": [D, R],
        "vr1": [128, DA],
        "vr2": [R - 128, DA],
        "vch": [128, NKC, DA],
        "vb": [BS, NB, DA],
    }

    def dram_slice(name, dmap, p):
        if name == "vr1":
            return dmap["vr"][p, 0:128, :]
        if name == "vr2":
            return dmap["vr"][p, 128:R, :]
        return dmap[name][p]

    # block chunking: 7 chunks of 8 blocks + 1 chunk of 7 blocks
    chunks = []
    n0 = 0
    while n0 < NB:
        nb = min(8, NB - n0)
        chunks.append((n0, nb))
        n0 += nb

    for p in range(NPAIR):
        # ---- load pair inputs (per required precision class) ----
        views = {}
        for name, dts in needs.items():
            v32 = v16 = None
            if any(d != bf16 for d in dts):
                v32 = pin.tile(shapes[name], DT32, tag=name + "32")
                nc.sync.dma_start(out=v32, in_=dram_slice(name, d32, p))
            if bf16 in dts:
                v16 = pin.tile(shapes[name], bf16, tag=name + "16")
                nc.sync.dma_start(out=v16, in_=dram_slice(name, d16, p))

            def mk(v32=v32, v16=v16):
                def get(dt):
                    if dt == bf16:
                        return v16
                    return as_dt(v32, dt)

                return get

            views[name] = mk()

        qt, kt, krt, vc, vbt = (
            views["qT"],
            views["kT"],
            views["krT"],
            views["vch"],
            views["vb"],
        )
        vr1, vr2 = views["vr1"], views["vr2"]

        # ---- Stage A: global queries (tokens 0..G) ----
        egt = pegt.tile([128, NKC, G], DT_SMALL, tag="egt")
        for kc in range(NKC):
            sgt = ps2.tile([128, G], f32, tag="big")
            nc.tensor.matmul(
                sgt,
                kt(DT_SMALL)[:, 128 * kc : 128 * (kc + 1)],
                qt(DT_SMALL)[:, 0:G],
                start=True,
                stop=True,
            )
            nc.scalar.activation(egt[:, kc, :], sgt, EXP, scale=INV_SCALE)
        outg = ps2.tile([DA, G], f32, tag="pt")
        for kc in range(NKC):
            nc.tensor.matmul(
                outg,
                vc(DT_SMALL)[:, kc, :],
                egt[:, kc, :],
                start=(kc == 0),
                stop=(kc == NKC - 1),
            )
        outg_sb = pe.tile([DA, G], f32, tag="poutsA")
        nc.vector.tensor_copy(outg_sb, outg)
        outg2 = ps2.tile([G, DA], f32, tag="pt")
        nc.tensor.transpose(outg2, outg_sb, ident[:DA, :DA])
        recg = pe.tile([G, 1], f32, tag="recipA")
        nc.vector.reciprocal(recg, outg2[:, D : D + 1])
        outg_n = pe.tile([G, D], f32, tag="outnA")
        nc.vector.tensor_scalar_mul(outg_n, outg2[:, 0:D], recg)
        nc.sync.dma_start(out=out[p, 0:G, :], in_=outg_n)

        # ---- Stage B: block queries ----
        for n0, nb in chunks:
            cq = BS * nb
            qoff = G + BS * n0  # first query token of this chunk

            ploc = ps1.tile([BS, 512], f32, tag="s_loc")
            pglo = ps1.tile([G, 512], f32, tag="s_glo")
            prnd = ps1.tile([128, 512], f32, tag="s_rnd")
            prnd2 = ps1.tile([R - 128, 512], f32, tag="s_rnd2")

            for j in range(nb):
                n = n0 + j
                koff = G + BS * n
                nc.tensor.matmul(
                    ploc[:, BS * j : BS * (j + 1)],
                    kt(DT_SMALL)[:, koff : koff + BS],
                    qt(DT_SMALL)[:, koff : koff + BS],
                    start=(j == 0),
                    stop=(j == nb - 1),
                )
            nc.tensor.matmul(
                pglo[:, 0:cq],
                kt(DT_BIG)[:, 0:G],
                qt(DT_BIG)[:, qoff : qoff + cq],
                start=True,
                stop=True,
            )
            nc.tensor.matmul(
                prnd[:, 0:cq],
                krt(DT_BIG)[:, 0:128],
                qt(DT_BIG)[:, qoff : qoff + cq],
                start=True,
                stop=True,
            )
            nc.tensor.matmul(
                prnd2[:, 0:cq],
                krt(DT_BIG)[:, 128:R],
                qt(DT_BIG)[:, qoff : qoff + cq],
                start=True,
                stop=True,
            )

            eloc = pe.tile([BS, 512], DT_SMALL, tag="eloc")
            eglo = pe.tile([G, 512], DT_BIG, tag="eglo")
            ernd = pe.tile([128, 512], DT_BIG, tag="ernd")
            ernd2 = pe.tile([R - 128, 512], DT_BIG, tag="ernd2")
            nc.scalar.activation(eloc[:, 0:cq], ploc[:, 0:cq], EXP, scale=INV_SCALE)
            nc.scalar.activation(eglo[:, 0:cq], pglo[:, 0:cq], EXP, scale=INV_SCALE)
            nc.scalar.activation(ernd[:, 0:cq], prnd[:, 0:cq], EXP, scale=INV_SCALE)
            nc.scalar.activation(ernd2[:, 0:cq], prnd2[:, 0:cq], EXP, scale=INV_SCALE)

            pout = ps2.tile([DA, 512], f32, tag="big")
            nc.tensor.matmul(
                pout[:, 0:cq],
                vc(DT_BIG)[0:G, 0, :],
                eglo[:, 0:cq],
                start=True,
                stop=False,
            )
            nc.tensor.matmul(
                pout[:, 0:cq],
                vr1(DT_BIG),
                ernd[:, 0:cq],
                start=False,
                stop=False,
            )
            nc.tensor.matmul(
                pout[:, 0:cq],
                vr2(DT_BIG),
                ernd2[:, 0:cq],
                start=False,
                stop=False,
            )
            for j in range(nb):
                n = n0 + j
                nc.tensor.matmul(
                    pout[:, BS * j : BS * (j + 1)],
                    vbt(DT_SMALL)[:, n, :],
                    eloc[:, BS * j : BS * (j + 1)],
                    start=False,
                    stop=(j == nb - 1),
                )

            pout_sb = pe.tile([DA, 512], f32, tag="pouts")
            nc.vector.tensor_copy(pout_sb[:, 0:cq], pout[:, 0:cq])

            nt = (cq + 127) // 128
            pt = ps2.tile([128, nt * DA], f32, tag="pt")
            for t in range(nt):
                w = min(128, cq - 128 * t)
                nc.tensor.matmul(
                    pt[0:w, DA * t : DA * (t + 1)],
                    pout_sb[:, 128 * t : 128 * t + w],
                    ident[:DA, :DA],
                    is_transpose=True,
                    start=True,
                    stop=True,
                )
            for t in range(nt):
                w = min(128, cq - 128 * t)
                rec = pe.tile([128, 1], f32, tag="recip")
                nc.vector.reciprocal(rec[0:w], pt[0:w, DA * t + D : DA * (t + 1)])
                outn = pe.tile([128, D], f32, tag="outn")
                nc.vector.tensor_scalar_mul(
                    outn[0:w], pt[0:w, DA * t : DA * t + D], rec[0:w]
                )
                r0 = qoff + 128 * t
                nc.sync.dma_start(out=out[p, r0 : r0 + w, :], in_=outn[0:w])


def _build_program():
    from contextlib import ExitStack

    import concourse.bacc as bacc
    import concourse.mybir as mybir
    import concourse.tile as tile

    DT32 = _dt32_main(mybir)
    bf16 = mybir.dt.bfloat16

    nc = bacc.Bacc(
        "TRN2", target_bir_lowering=False, debug=False, num_devices=NCORE
    )
    shapes = {
        "qT": [NPAIR, D, T],
        "kT": [NPAIR, D, T],
        "krT": [NPAIR, D, R],
        "vch": [NPAIR, 128, NKC, DA],
        "vr": [NPAIR, R, DA],
        "vb": [NPAIR, BS, NB, DA],
    }
    d32 = {
        name: nc.dram_tensor(name + "32", shp, DT32, kind="ExternalInput").ap()
        for name, shp in shapes.items()
    }
    d16 = {
        name: nc.dram_tensor(name + "16", shp, bf16, kind="ExternalInput").ap()
        for name, shp in shapes.items()
    }
    out = nc.dram_tensor(
        "out", [NPAIR, T, D], mybir.dt.float32, kind="ExternalOutput"
    ).ap()

    with tile.TileContext(nc) as tc:
        with ExitStack() as ctx:
            _body(ctx, tc, d32, d16, out)
    nc.compile()
    return nc


def get_program():
    key = (BIG_MM, SMALL_MM)
    if key not in _PROGRAM_CACHE:
        _PROGRAM_CACHE[key] = _build_program()
    return _PROGRAM_CACHE[key]


def prep_inputs(q, k, v, rand_idx):
    """Host-side shard + layout prep. Returns list of per-core input dicts."""
    import ml_dtypes

    bf16 = ml_dtypes.bfloat16
    idx = np.asarray(rand_idx).astype(np.int64)
    qp = np.ascontiguousarray(q.transpose(0, 2, 3, 1)).reshape(BH, D, T)
    kp = np.ascontiguousarray(k.transpose(0, 2, 3, 1)).reshape(BH, D, T)
    krTp = np.ascontiguousarray(kp[:, :, idx])  # [BH, D, R]
    vp = np.ascontiguousarray(v.transpose(0, 2, 1, 3)).reshape(BH, T, D)
    v_aug = np.concatenate([vp, np.ones((BH, T, 1), np.float32)], axis=2)
    vchp = np.ascontiguousarray(
        v_aug.reshape(BH, NKC, 128, DA).transpose(0, 2, 1, 3)
    )  # [BH, 128, NKC, DA]
    vrp = np.ascontiguousarray(v_aug[:, idx, :])  # [BH, R, DA]
    vbp = np.ascontiguousarray(
        v_aug[:, G:, :].reshape(BH, NB, BS, DA).transpose(0, 2, 1, 3)
    )  # [BH, BS, NB, DA]

    full = {
        "qT": qp,
        "kT": kp,
        "krT": krTp,
        "vch": vchp,
        "vr": vrp,
        "vb": vbp,
    }
    in_maps = []
    for c in range(NCORE):
        s = slice(c * NPAIR, (c + 1) * NPAIR)
        m = {}
        for name, arr in full.items():
            part = np.ascontiguousarray(arr[s])
            m[name + "32"] = part
            m[name + "16"] = part.astype(bf16)
        in_maps.append(m)
    return in_maps


def assemble_output(results):
    """[8 cores] x {"out": [NPAIR, T, D]} -> [B, T, H, D]"""
    full = np.concatenate([r["out"] for r in results], axis=0)  # [BH, T, D]
    return np.ascontiguousarray(
        full.reshape(B, H, T, D).transpose(0, 2, 1, 3)
    )


def kernel(q, k, v, rand_idx, _trace=False):
    from concourse.bass_utils import run_bass_kernel_spmd

    nc = get_program()
    in_maps = prep_inputs(
        np.asarray(q, dtype=np.float32),
        np.asarray(k, dtype=np.float32),
        np.asarray(v, dtype=np.float32),
        rand_idx,
    )
    res = run_bass_kernel_spmd(nc, in_maps, list(range(NCORE)), trace=_trace)
    out = assemble_output(res.results)
    if _trace:
        return out, res
    return out


# revision 10
# speedup vs baseline: 1.5512x; 1.0405x over previous
"""BigBird sparse attention on 8 Trainium2 NeuronCores.

Sharding: batch*heads = 64 (b,h) pairs, 8 pairs per core (pure data
parallel, no collectives). Host-side prep transposes Q/K to [D, T],
gathers the random keys/values, and augments V with a ones column so the
softmax denominator falls out of the attention*V matmul as row D.

Per (b,h) pair on-core:
  Stage A (global queries, tokens 0..63): S_g^T chunks [128k, 64q] via
    matmul(lhsT=K^T chunk, rhs=Qg^T), exp on ScalarE, then accumulated
    AV matmuls with V_aug chunks as stationary producing out_g^T [65, 64]
    (row 64 = denominator). PE-transpose back, normalize on VectorE.
  Stage B (block queries): chunks of up to 8 blocks (512 queries).
    Scores are computed keys-on-partitions: local per block [64, 64],
    global [64, CQ], random [128+64, CQ]. exp on ScalarE -> E tiles in
    SBUF. AV: out^T [65, CQ] accumulating matmuls with V_aug stationary
    (global/random batched over the whole chunk, local per block).
    PE-transpose to [q, 65], normalize by reciprocal of col 64, DMA out.

Softmax skips max-subtraction: scores/sqrt(D) are ~N(0,1) for randn
inputs, so exp stays comfortably inside fp32 range and normalization
cancels the shift exactly in exact math.

Precision knobs BIG_MM / SMALL_MM select matmul input dtype per class.
float32r data must be produced as float32r end-to-end; bf16 operands are
prepared host-side and exp() writes bf16 E tiles directly.
"""

import numpy as np

B, T, H, D = 4, 4096, 16, 64
BS, G, R = 64, 64, 192
NCORE = 8
BH = B * H
NPAIR = BH // NCORE          # 8 pairs per core
NB = (T - G) // BS           # 63 local blocks
INV_SCALE = float(D) ** -0.5
DA = D + 1                   # V augmented with ones column
NKC = T // 128               # 32 key chunks of 128

# matmul input precision for the large-N (>=256 moving) matmuls:
#   "f32"  - exact fp32 (4 cycles/row)
#   "f32r" - hardware fp32r (reduced precision, ~2 passes on HW)
#   "bf16" - bfloat16 inputs, fp32 accumulate (1 cycle/row, single pass)
BIG_MM = "bf16"
# precision for the small-N (64/65 moving) matmuls: "f32", "f32r" or "bf16"
SMALL_MM = "bf16"

_PROGRAM_CACHE = {}


def _dt32_main(mybir):
    """dtype used for the 32-bit DRAM/SBUF copies."""
    if "f32r" in (BIG_MM, SMALL_MM):
        return mybir.dt.float32r
    return mybir.dt.float32


def _body(ctx, tc, d32, d16, out):
    import concourse.mybir as mybir
    from concourse.masks import make_identity

    nc = tc.nc
    f32 = mybir.dt.float32
    f32r = mybir.dt.float32r
    bf16 = mybir.dt.bfloat16
    DT32 = _dt32_main(mybir)
    DT = {"f32": f32, "f32r": f32r, "bf16": bf16}
    DT_BIG = DT[BIG_MM]
    DT_SMALL = DT[SMALL_MM]
    EXP = mybir.ActivationFunctionType.Exp

    def as_dt(ap, dt):
        return ap if ap.dtype == dt else ap.bitcast(dt)

    consts = ctx.enter_context(tc.tile_pool(name="consts", bufs=1))
    pin = ctx.enter_context(tc.tile_pool(name="pin", bufs=2))
    pe = ctx.enter_context(tc.tile_pool(name="pe", bufs=3))
    pegt = ctx.enter_context(tc.tile_pool(name="pegt", bufs=2))
    ps1 = ctx.enter_context(tc.tile_pool(name="ps1", bufs=1, space="PSUM"))
    ps2 = ctx.enter_context(tc.tile_pool(name="ps2", bufs=2, space="PSUM"))

    ident = consts.tile([128, 128], f32)
    make_identity(nc, ident)

    needs = {
        "qT": {DT_BIG, DT_SMALL},
        "kT": {DT_BIG, DT_SMALL},
        "krT": {DT_BIG},
        "vr1": {DT_BIG},
        "vr2": {DT_BIG},
        "vch": {DT_BIG, DT_SMALL},
        "vb": {DT_SMALL},
    }
    shapes = {
        "qT": [D, T],
        "kT": [D, T],
        "krT": [D, R + 1],
        "vr1": [128, DA],
        "vr2": [R - 128, DA],
        "vch": [128, NKC, DA],
        "vb": [BS, NB, DA],
    }

    def dram_slice(name, dmap, p):
        if name == "vr1":
            return dmap["vr"][p, 0:128, :]
        if name == "vr2":
            return dmap["vr"][p, 128:R, :]
        return dmap[name][p]

    # block chunking: 7 chunks of 8 blocks + 1 chunk of 7 blocks
    chunks = []
    n0 = 0
    while n0 < NB:
        nb = min(8, NB - n0)
        chunks.append((n0, nb))
        n0 += nb

    for p in range(NPAIR):
        # ---- load pair inputs (per required precision class) ----
        views = {}
        for name, dts in needs.items():
            v32 = v16 = None
            if any(d != bf16 for d in dts):
                v32 = pin.tile(shapes[name], DT32, tag=name + "32")
                nc.sync.dma_start(out=v32, in_=dram_slice(name, d32, p))
            if bf16 in dts:
                v16 = pin.tile(shapes[name], bf16, tag=name + "16")
                nc.sync.dma_start(out=v16, in_=dram_slice(name, d16, p))

            def mk(v32=v32, v16=v16):
                def get(dt):
                    if dt == bf16:
                        return v16
                    return as_dt(v32, dt)

                return get

            views[name] = mk()

        qt, kt, krt, vc, vbt = (
            views["qT"],
            views["kT"],
            views["krT"],
            views["vch"],
            views["vb"],
        )
        vr1, vr2 = views["vr1"], views["vr2"]

        # ---- Stage A: global queries (tokens 0..G) ----
        egt = pegt.tile([128, NKC, G], DT_SMALL, tag="egt")
        for kc in range(NKC):
            sgt = ps2.tile([128, G], f32, tag="big")
            nc.tensor.matmul(
                sgt,
                kt(DT_SMALL)[:, 128 * kc : 128 * (kc + 1)],
                qt(DT_SMALL)[:, 0:G],
                start=True,
                stop=True,
            )
            nc.scalar.activation(egt[:, kc, :], sgt, EXP, scale=INV_SCALE)
        outg = ps2.tile([DA, G], f32, tag="pt")
        for kc in range(NKC):
            nc.tensor.matmul(
                outg,
                vc(DT_SMALL)[:, kc, :],
                egt[:, kc, :],
                start=(kc == 0),
                stop=(kc == NKC - 1),
            )
        outg_sb = pe.tile([DA, G], f32, tag="poutsA")
        nc.vector.tensor_copy(outg_sb, outg)
        outg2 = ps2.tile([G, DA], f32, tag="pt")
        nc.tensor.transpose(outg2, outg_sb, ident[:DA, :DA])
        recg = pe.tile([G, 1], f32, tag="recipA")
        nc.vector.reciprocal(recg, outg2[:, D : D + 1])
        outg_n = pe.tile([G, D], f32, tag="outnA")
        nc.vector.tensor_scalar_mul(outg_n, outg2[:, 0:D], recg)
        nc.sync.dma_start(out=out[p, 0:G, :], in_=outg_n)

        # ---- Stage B: block queries ----
        for n0, nb in chunks:
            cq = BS * nb
            qoff = G + BS * n0  # first query token of this chunk

            ploc = ps1.tile([BS + 1, 512], f32, tag="s_loc")
            pglo = ps1.tile([G + 1, 512], f32, tag="s_glo")
            prnd = ps1.tile([128, 512], f32, tag="s_rnd")
            prnd2 = ps1.tile([R - 128 + 1, 512], f32, tag="s_rnd2")

            # issue order: blocks whose stationary can be 65 wide must
            # open and close the psum group (uniform partition coverage)
            js = list(range(nb))
            defic = [j for j in js if G + BS * (n0 + j) + BS + 1 > T]
            full = [j for j in js if j not in defic]
            order = [full[0]] + defic + full[1:]
            for oi, j in enumerate(order):
                n = n0 + j
                koff = G + BS * n
                mw = BS + 1 if j not in defic else BS
                nc.tensor.matmul(
                    ploc[0:mw, BS * j : BS * (j + 1)],
                    kt(DT_SMALL)[:, koff : koff + mw],
                    qt(DT_SMALL)[:, koff : koff + BS],
                    start=(oi == 0),
                    stop=(oi == len(order) - 1),
                )
            nc.tensor.matmul(
                pglo[:, 0:cq],
                kt(DT_BIG)[:, 0 : G + 1],
                qt(DT_BIG)[:, qoff : qoff + cq],
                start=True,
                stop=True,
            )
            nc.tensor.matmul(
                prnd[:, 0:cq],
                krt(DT_BIG)[:, 0:128],
                qt(DT_BIG)[:, qoff : qoff + cq],
                start=True,
                stop=True,
            )
            nc.tensor.matmul(
                prnd2[:, 0:cq],
                krt(DT_BIG)[:, 128 : R + 1],
                qt(DT_BIG)[:, qoff : qoff + cq],
                start=True,
                stop=True,
            )

            eloc = pe.tile([BS, 512], DT_SMALL, tag="eloc")
            eglo = pe.tile([G, 512], DT_BIG, tag="eglo")
            ernd = pe.tile([128, 512], DT_BIG, tag="ernd")
            ernd2 = pe.tile([R - 128, 512], DT_BIG, tag="ernd2")
            nc.scalar.activation(eloc[:, 0:cq], ploc[0:BS, 0:cq], EXP, scale=INV_SCALE)
            nc.scalar.activation(eglo[:, 0:cq], pglo[0:G, 0:cq], EXP, scale=INV_SCALE)
            nc.scalar.activation(ernd[:, 0:cq], prnd[:, 0:cq], EXP, scale=INV_SCALE)
            nc.scalar.activation(ernd2[:, 0:cq], prnd2[0 : R - 128, 0:cq], EXP, scale=INV_SCALE)

            pout = ps2.tile([DA, 512], f32, tag="big")
            nc.tensor.matmul(
                pout[:, 0:cq],
                vc(DT_BIG)[0:G, 0, :],
                eglo[:, 0:cq],
                start=True,
                stop=False,
            )
            nc.tensor.matmul(
                pout[:, 0:cq],
                vr1(DT_BIG),
                ernd[:, 0:cq],
                start=False,
                stop=False,
            )
            nc.tensor.matmul(
                pout[:, 0:cq],
                vr2(DT_BIG),
                ernd2[:, 0:cq],
                start=False,
                stop=False,
            )
            for j in range(nb):
                n = n0 + j
                nc.tensor.matmul(
                    pout[:, BS * j : BS * (j + 1)],
                    vbt(DT_SMALL)[:, n, :],
                    eloc[:, BS * j : BS * (j + 1)],
                    start=False,
                    stop=(j == nb - 1),
                )

            pout_sb = pe.tile([DA, 512], f32, tag="pouts")
            nc.vector.tensor_copy(pout_sb[:, 0:cq], pout[:, 0:cq])

            nt = (cq + 127) // 128
            pt = ps2.tile([128, nt * DA], f32, tag="pt")
            for t in range(nt):
                w = min(128, cq - 128 * t)
                nc.tensor.matmul(
                    pt[0:w, DA * t : DA * (t + 1)],
                    pout_sb[:, 128 * t : 128 * t + w],
                    ident[:DA, :DA],
                    is_transpose=True,
                    start=True,
                    stop=True,
                )
            for t in range(nt):
                w = min(128, cq - 128 * t)
                rec = pe.tile([128, 1], f32, tag="recip")
                nc.vector.reciprocal(rec[0:w], pt[0:w, DA * t + D : DA * (t + 1)])
                outn = pe.tile([128, D], f32, tag="outn")
                nc.vector.tensor_scalar_mul(
                    outn[0:w], pt[0:w, DA * t : DA * t + D], rec[0:w]
                )
                r0 = qoff + 128 * t
                nc.sync.dma_start(out=out[p, r0 : r0 + w, :], in_=outn[0:w])


def _build_program():
    from contextlib import ExitStack

    import concourse.bacc as bacc
    import concourse.mybir as mybir
    import concourse.tile as tile

    DT32 = _dt32_main(mybir)
    bf16 = mybir.dt.bfloat16

    nc = bacc.Bacc(
        "TRN2", target_bir_lowering=False, debug=False, num_devices=NCORE
    )
    shapes = {
        "qT": [NPAIR, D, T],
        "kT": [NPAIR, D, T],
        "krT": [NPAIR, D, R + 1],
        "vch": [NPAIR, 128, NKC, DA],
        "vr": [NPAIR, R, DA],
        "vb": [NPAIR, BS, NB, DA],
    }
    d32 = {
        name: nc.dram_tensor(name + "32", shp, DT32, kind="ExternalInput").ap()
        for name, shp in shapes.items()
    }
    d16 = {
        name: nc.dram_tensor(name + "16", shp, bf16, kind="ExternalInput").ap()
        for name, shp in shapes.items()
    }
    out = nc.dram_tensor(
        "out", [NPAIR, T, D], mybir.dt.float32, kind="ExternalOutput"
    ).ap()

    with tile.TileContext(nc) as tc:
        with ExitStack() as ctx:
            _body(ctx, tc, d32, d16, out)
    nc.compile()
    return nc


def get_program():
    key = (BIG_MM, SMALL_MM)
    if key not in _PROGRAM_CACHE:
        _PROGRAM_CACHE[key] = _build_program()
    return _PROGRAM_CACHE[key]


def prep_inputs(q, k, v, rand_idx):
    """Host-side shard + layout prep. Returns list of per-core input dicts."""
    import ml_dtypes

    bf16 = ml_dtypes.bfloat16
    idx = np.asarray(rand_idx).astype(np.int64)
    qp = np.ascontiguousarray(q.transpose(0, 2, 3, 1)).reshape(BH, D, T)
    kp = np.ascontiguousarray(k.transpose(0, 2, 3, 1)).reshape(BH, D, T)
    krTp = np.ascontiguousarray(
        np.concatenate([kp[:, :, idx], np.zeros((BH, D, 1), np.float32)], axis=2)
    )  # [BH, D, R+1]
    vp = np.ascontiguousarray(v.transpose(0, 2, 1, 3)).reshape(BH, T, D)
    v_aug = np.concatenate([vp, np.ones((BH, T, 1), np.float32)], axis=2)
    vchp = np.ascontiguousarray(
        v_aug.reshape(BH, NKC, 128, DA).transpose(0, 2, 1, 3)
    )  # [BH, 128, NKC, DA]
    vrp = np.ascontiguousarray(v_aug[:, idx, :])  # [BH, R, DA]
    vbp = np.ascontiguousarray(
        v_aug[:, G:, :].reshape(BH, NB, BS, DA).transpose(0, 2, 1, 3)
    )  # [BH, BS, NB, DA]

    full = {
        "qT": qp,
        "kT": kp,
        "krT": krTp,
        "vch": vchp,
        "vr": vrp,
        "vb": vbp,
    }
    in_maps = []
    for c in range(NCORE):
        s = slice(c * NPAIR, (c + 1) * NPAIR)
        m = {}
        for name, arr in full.items():
            part = np.ascontiguousarray(arr[s])
            m[name + "32"] = part
            m[name + "16"] = part.astype(bf16)
        in_maps.append(m)
    return in_maps


def assemble_output(results):
    """[8 cores] x {"out": [NPAIR, T, D]} -> [B, T, H, D]"""
    full = np.concatenate([r["out"] for r in results], axis=0)  # [BH, T, D]
    return np.ascontiguousarray(
        full.reshape(B, H, T, D).transpose(0, 2, 1, 3)
    )


def kernel(q, k, v, rand_idx, _trace=False):
    from concourse.bass_utils import run_bass_kernel_spmd

    nc = get_program()
    in_maps = prep_inputs(
        np.asarray(q, dtype=np.float32),
        np.asarray(k, dtype=np.float32),
        np.asarray(v, dtype=np.float32),
        rand_idx,
    )
    res = run_bass_kernel_spmd(nc, in_maps, list(range(NCORE)), trace=_trace)
    out = assemble_output(res.results)
    if _trace:
        return out, res
    return out


# revision 13
# speedup vs baseline: 1.6466x; 1.0615x over previous
"""BigBird sparse attention on 8 Trainium2 NeuronCores.

Sharding: batch*heads = 64 (b,h) pairs, 8 per core (data parallel, no
collectives). On-core, pairs are processed two at a time ("A"/"B")
stacked on SBUF partitions 0-63 / 64-127: with tile_position quadrant
packing the PE runs A's and B's matmuls concurrently in disjoint
regions of the 128x128 array, and ScalarE exp() always sees full-height
[128, x] tiles.

Host prep: Q/K transposed to [D, T]; random K/V gathered by rand_idx;
V augmented with a ones column (so the softmax denominator falls out of
the AV matmul as row D); V variants stacked per pair-duo.

Per pair-duo:
  Stage A (global queries 0..63): S_g^T chunks [128k, 64q] per pair
    (B's matmuls at tile_position=(64,0), row-concurrent with A's),
    exp, then K=128 accumulated AV matmuls -> out_g^T [65, 64],
    PE-transpose back, normalize, DMA.
  Stage B (63 local blocks in chunks of <=8 blocks / 512 queries):
    scores keys-on-partitions in 5 full-height PSUM tiles
    {local, global, rnd0, rnd1, rnd2} with A in partitions 0-63 and B
    in 64-127 (B at tile_position=(64,64)); one exp per tile; AV into
    per-pair out^T [65, cq] accumulators (B at tile_position=(64,0));
    PE-transpose to [q, 65]; normalize by reciprocal of column 64; DMA.

Softmax skips max-subtraction: scores/sqrt(D) are ~N(0,1) for randn
inputs, so exp stays comfortably inside fp32 range and normalization
cancels the shift exactly in exact math.

All matmul inputs are bf16 (fp32 PSUM accumulation); everything after
exp stays fp32 through normalization.
"""

import numpy as np

B, T, H, D = 4, 4096, 16, 64
BS, G, R = 64, 64, 192
NCORE = 8
BH = B * H
NPAIR = BH // NCORE          # 8 pairs per core
NSP = NPAIR // 2             # 4 stacked pair-duos per core
NB = (T - G) // BS           # 63 local blocks
INV_SCALE = float(D) ** -0.5
DA = D + 1                   # V augmented with ones column
NKC = T // 128               # 32 key chunks of 128
NR = R // BS                 # 3 random-key groups of 64

_PROGRAM_CACHE = {}


def _body(ctx, tc, din, out):
    import concourse.mybir as mybir
    from concourse.masks import make_identity

    nc = tc.nc
    f32 = mybir.dt.float32
    bf16 = mybir.dt.bfloat16
    EXP = mybir.ActivationFunctionType.Exp

    consts = ctx.enter_context(tc.tile_pool(name="consts", bufs=1))
    pin = ctx.enter_context(tc.tile_pool(name="pin", bufs=2))
    pe = ctx.enter_context(tc.tile_pool(name="pe", bufs=3))
    pegt = ctx.enter_context(tc.tile_pool(name="pegt", bufs=2))
    psS = ctx.enter_context(tc.tile_pool(name="psS", bufs=1, space="PSUM"))
    psO = ctx.enter_context(tc.tile_pool(name="psO", bufs=1, space="PSUM"))
    psT = ctx.enter_context(tc.tile_pool(name="psT", bufs=1, space="PSUM"))

    ident = consts.tile([128, 128], f32)
    make_identity(nc, ident)

    # block chunking: 7 chunks of 8 blocks + 1 chunk of 7 blocks
    chunks = []
    n0 = 0
    while n0 < NB:
        nb = min(8, NB - n0)
        chunks.append((n0, nb))
        n0 += nb

    halves = ((0, slice(0, 64)), (1, slice(64, 128)))

    for sp in range(NSP):
        pA, pB = 2 * sp, 2 * sp + 1
        # ---- load stacked inputs ----
        qt2 = pin.tile([128, T], bf16, tag="qt2")
        kt2 = pin.tile([128, T], bf16, tag="kt2")
        krt2 = pin.tile([128, R], bf16, tag="krt2")
        vg2 = pin.tile([128, DA], bf16, tag="vg2")
        vr2 = pin.tile([128, NR, DA], bf16, tag="vr2")
        vb2 = pin.tile([128, NB, DA], bf16, tag="vb2")
        vcA = pin.tile([128, NKC, DA], bf16, tag="vcA")
        vcB = pin.tile([128, NKC, DA], bf16, tag="vcB")
        nc.sync.dma_start(out=qt2[0:64, :], in_=din["qT"][pA])
        nc.sync.dma_start(out=qt2[64:128, :], in_=din["qT"][pB])
        nc.sync.dma_start(out=kt2[0:64, :], in_=din["kT"][pA])
        nc.sync.dma_start(out=kt2[64:128, :], in_=din["kT"][pB])
        nc.sync.dma_start(out=krt2[0:64, :], in_=din["krT"][pA])
        nc.sync.dma_start(out=krt2[64:128, :], in_=din["krT"][pB])
        nc.sync.dma_start(out=vg2, in_=din["vgs"][sp])
        nc.sync.dma_start(out=vr2, in_=din["vrs"][sp])
        nc.sync.dma_start(out=vb2, in_=din["vbs"][sp])
        nc.sync.dma_start(out=vcA, in_=din["vch"][pA])
        nc.sync.dma_start(out=vcB, in_=din["vch"][pB])

        # ---- Stage A: global queries ----
        egtA = pegt.tile([128, NKC, G], bf16, tag="egtA")
        egtB = pegt.tile([128, NKC, G], bf16, tag="egtB")
        for kc in range(NKC):
            sgtA = psS.tile([128, G], f32, tag="s_loc")
            sgtB = psS.tile([128, G], f32, tag="s_glo")
            nc.tensor.matmul(
                sgtA,
                kt2[0:64, 128 * kc : 128 * (kc + 1)],
                qt2[0:64, 0:G],
                start=True,
                stop=True,
            )
            nc.tensor.matmul(
                sgtB,
                kt2[64:128, 128 * kc : 128 * (kc + 1)],
                qt2[64:128, 0:G],
                tile_position=(64, 0),
                start=True,
                stop=True,
            )
            nc.scalar.activation(egtA[:, kc, :], sgtA, EXP, scale=INV_SCALE)
            nc.scalar.activation(egtB[:, kc, :], sgtB, EXP, scale=INV_SCALE)
        outgA = psO.tile([DA, G], f32, tag="pout_A")
        outgB = psO.tile([DA, G], f32, tag="pout_B")
        for kc in range(NKC):
            nc.tensor.matmul(
                outgA,
                vcA[:, kc, :],
                egtA[:, kc, :],
                start=(kc == 0),
                stop=(kc == NKC - 1),
            )
            nc.tensor.matmul(
                outgB,
                vcB[:, kc, :],
                egtB[:, kc, :],
                start=(kc == 0),
                stop=(kc == NKC - 1),
            )
        for p, outg in ((pA, outgA), (pB, outgB)):
            outg_sb = pe.tile([DA, G], f32, tag="poutsA")
            nc.vector.tensor_copy(outg_sb, outg)
            outg2 = psT.tile([G, DA], f32, tag="pt")
            nc.tensor.transpose(outg2, outg_sb, ident[:DA, :DA])
            recg = pe.tile([G, 1], f32, tag="recipA")
            nc.vector.reciprocal(recg, outg2[:, D : D + 1])
            outg_n = pe.tile([G, D], f32, tag="outnA")
            nc.vector.tensor_scalar_mul(outg_n, outg2[:, 0:D], recg)
            nc.sync.dma_start(out=out[p, 0:G, :], in_=outg_n)

        # ---- Stage B: block queries ----
        for n0, nb in chunks:
            cq = BS * nb
            qoff = G + BS * n0

            ploc = psS.tile([128, 512], f32, tag="s_loc")
            pglo = psS.tile([128, 512], f32, tag="s_glo")
            prnd = [
                psS.tile([128, 512], f32, tag=f"s_r{j}", name=f"prnd{j}")
                for j in range(NR)
            ]

            # Two waves so same-bank A/B groups never interleave, while
            # adjacent instructions still hit disjoint array quadrants:
            # wave 1: A-local stream ||| B-global/random; wave 2 swapped.
            def s_loc_mm(hi, rows, j):
                koff = G + BS * (n0 + j)
                nc.tensor.matmul(
                    ploc[rows, BS * j : BS * (j + 1)],
                    kt2[rows, koff : koff + BS],
                    qt2[rows, koff : koff + BS],
                    tile_position=(64, 64) if hi else None,
                    start=(j == 0),
                    stop=(j == nb - 1),
                )

            def s_big_mms(hi, rows):
                tp = (64, 64) if hi else None
                yield lambda: nc.tensor.matmul(
                    pglo[rows, 0:cq],
                    kt2[rows, 0:G],
                    qt2[rows, qoff : qoff + cq],
                    tile_position=tp,
                    start=True,
                    stop=True,
                )
                for j in range(NR):
                    yield (
                        lambda j=j: nc.tensor.matmul(
                            prnd[j][rows, 0:cq],
                            krt2[rows, BS * j : BS * (j + 1)],
                            qt2[rows, qoff : qoff + cq],
                            tile_position=tp,
                            start=True,
                            stop=True,
                        )
                    )

            for wave in range(2):
                hi_loc, rows_loc = halves[wave]
                hi_big, rows_big = halves[1 - wave]
                big = list(s_big_mms(hi_big, rows_big))
                for j in range(nb):
                    s_loc_mm(hi_loc, rows_loc, j)
                    if j < len(big):
                        big[j]()

            eloc = pe.tile([128, 512], bf16, tag="eloc")
            eglo = pe.tile([128, 512], bf16, tag="eglo")
            ernd = [
                pe.tile([128, 512], bf16, tag=f"er{j}", name=f"ernd{j}")
                for j in range(NR)
            ]
            nc.scalar.activation(eloc[:, 0:cq], ploc[:, 0:cq], EXP, scale=INV_SCALE)
            nc.scalar.activation(eglo[:, 0:cq], pglo[:, 0:cq], EXP, scale=INV_SCALE)
            for j in range(NR):
                nc.scalar.activation(
                    ernd[j][:, 0:cq], prnd[j][:, 0:cq], EXP, scale=INV_SCALE
                )

            poutA = psO.tile([DA, 512], f32, tag="pout_A")
            poutB = psO.tile([DA, 512], f32, tag="pout_B")
            pouts = (poutA, poutB)
            for hi, rows in halves:
                nc.tensor.matmul(
                    pouts[hi][:, 0:cq],
                    vg2[rows, :],
                    eglo[rows, 0:cq],
                    tile_position=(64, 0) if hi else None,
                    start=True,
                    stop=False,
                )
            for j in range(NR):
                for hi, rows in halves:
                    nc.tensor.matmul(
                        pouts[hi][:, 0:cq],
                        vr2[rows, j, :],
                        ernd[j][rows, 0:cq],
                        tile_position=(64, 0) if hi else None,
                        start=False,
                        stop=False,
                    )
            for j in range(nb):
                n = n0 + j
                for hi, rows in halves:
                    nc.tensor.matmul(
                        pouts[hi][:, BS * j : BS * (j + 1)],
                        vb2[rows, n, :],
                        eloc[rows, BS * j : BS * (j + 1)],
                        tile_position=(64, 0) if hi else None,
                        start=False,
                        stop=(j == nb - 1),
                    )

            nt = (cq + 127) // 128
            for hi, p in ((0, pA), (1, pB)):
                pout_sb = pe.tile([DA, 512], f32, tag="pouts")
                nc.vector.tensor_copy(pout_sb[:, 0:cq], pouts[hi][:, 0:cq])
                pt = psT.tile([128, nt * DA], f32, tag="pt")
                for t in range(nt):
                    w = min(128, cq - 128 * t)
                    nc.tensor.matmul(
                        pt[0:w, DA * t : DA * (t + 1)],
                        pout_sb[:, 128 * t : 128 * t + w],
                        ident[:DA, :DA],
                        is_transpose=True,
                        start=True,
                        stop=True,
                    )
                for t in range(nt):
                    w = min(128, cq - 128 * t)
                    rec = pe.tile([128, 1], f32, tag="recip")
                    nc.vector.reciprocal(
                        rec[0:w], pt[0:w, DA * t + D : DA * (t + 1)]
                    )
                    outn = pe.tile([128, D], f32, tag="outn")
                    nc.vector.tensor_scalar_mul(
                        outn[0:w], pt[0:w, DA * t : DA * t + D], rec[0:w]
                    )
                    r0 = qoff + 128 * t
                    nc.sync.dma_start(out=out[p, r0 : r0 + w, :], in_=outn[0:w])


def _build_program():
    from contextlib import ExitStack

    import concourse.bacc as bacc
    import concourse.mybir as mybir
    import concourse.tile as tile

    bf16 = mybir.dt.bfloat16
    nc = bacc.Bacc(
        "TRN2", target_bir_lowering=False, debug=False, num_devices=NCORE
    )
    shapes = {
        "qT": [NPAIR, D, T],
        "kT": [NPAIR, D, T],
        "krT": [NPAIR, D, R],
        "vch": [NPAIR, 128, NKC, DA],
        "vgs": [NSP, 128, DA],
        "vrs": [NSP, 128, NR, DA],
        "vbs": [NSP, 128, NB, DA],
    }
    din = {
        name: nc.dram_tensor(name, shp, bf16, kind="ExternalInput").ap()
        for name, shp in shapes.items()
    }
    out = nc.dram_tensor(
        "out", [NPAIR, T, D], mybir.dt.float32, kind="ExternalOutput"
    ).ap()

    with tile.TileContext(nc) as tc:
        with ExitStack() as ctx:
            _body(ctx, tc, din, out)
    nc.compile()
    return nc


def get_program():
    if "v3" not in _PROGRAM_CACHE:
        _PROGRAM_CACHE["v3"] = _build_program()
    return _PROGRAM_CACHE["v3"]


def prep_inputs(q, k, v, rand_idx):
    """Host-side shard + layout prep. Returns list of per-core input dicts."""
    import ml_dtypes

    bf16 = ml_dtypes.bfloat16
    idx = np.asarray(rand_idx).astype(np.int64)
    qp = np.ascontiguousarray(q.transpose(0, 2, 3, 1)).reshape(BH, D, T)
    kp = np.ascontiguousarray(k.transpose(0, 2, 3, 1)).reshape(BH, D, T)
    krTp = np.ascontiguousarray(kp[:, :, idx])  # [BH, D, R]
    vp = np.ascontiguousarray(v.transpose(0, 2, 1, 3)).reshape(BH, T, D)
    v_aug = np.concatenate([vp, np.ones((BH, T, 1), np.float32)], axis=2)
    vchp = np.ascontiguousarray(
        v_aug.reshape(BH, NKC, 128, DA).transpose(0, 2, 1, 3)
    )  # [BH, 128, NKC, DA]
    # stacked pair-duo variants: rows 0-63 = pair A, rows 64-127 = pair B
    vgs = v_aug[:, 0:G, :].reshape(BH // 2, 128, DA)
    vr = v_aug[:, idx, :].reshape(BH // 2, 2, NR, BS, DA)  # [sp, ab, j, key, da]
    vrs = np.ascontiguousarray(
        vr.transpose(0, 1, 3, 2, 4).reshape(BH // 2, 128, NR, DA)
    )
    vbs = np.ascontiguousarray(
        v_aug[:, G:, :].reshape(BH, NB, BS, DA).transpose(0, 2, 1, 3)
    ).reshape(BH // 2, 128, NB, DA)

    full = {
        "qT": qp,
        "kT": kp,
        "krT": krTp,
        "vch": vchp,
        "vgs": vgs,
        "vrs": vrs,
        "vbs": vbs,
    }
    in_maps = []
    for c in range(NCORE):
        m = {}
        for name, arr in full.items():
            per = arr.shape[0] // NCORE
            m[name] = np.ascontiguousarray(arr[c * per : (c + 1) * per]).astype(
                bf16
            )
        in_maps.append(m)
    return in_maps


def assemble_output(results):
    """[8 cores] x {"out": [NPAIR, T, D]} -> [B, T, H, D]"""
    full = np.concatenate([r["out"] for r in results], axis=0)  # [BH, T, D]
    return np.ascontiguousarray(
        full.reshape(B, H, T, D).transpose(0, 2, 1, 3)
    )


def kernel(q, k, v, rand_idx, _trace=False):
    from concourse.bass_utils import run_bass_kernel_spmd

    nc = get_program()
    in_maps = prep_inputs(
        np.asarray(q, dtype=np.float32),
        np.asarray(k, dtype=np.float32),
        np.asarray(v, dtype=np.float32),
        rand_idx,
    )
    res = run_bass_kernel_spmd(nc, in_maps, list(range(NCORE)), trace=_trace)
    out = assemble_output(res.results)
    if _trace:
        return out, res
    return out
